# revision 1
# baseline (speedup 1.0000x reference)
"""FlowNet-C correlation layer (MAX_DISP=20, STRIDE=2) on 8 trn2 cores.

Strategy: shard by (batch b, output-row half). Core k handles b=k//2,
output rows [24*(k%2), 24*(k%2)+24). Contraction over C=128 runs on the
TensorEngine as banded-Gram matmuls: per output row pair (h0,h0+1) and
dy-triple g, G2[w, col] += x1_row[128,96]^T @ x2p_rows[128,408] with the
h-pair pooled via PSUM accumulation (f32r, full rate at moving>=256).

The Gram is evicted PSUM->SBUF into a [w, col, dy] layout (dy
innermost) in bf16 by 2-bank-granular copies alternating DVE/Act
(psum bufs=4 keeps the PE streaming), then written to DRAM split over
the SP and Pool DMA queues. In that layout the 1681 wanted elements
(dx, dy) for one output pixel are a single contiguous 3362B run, so
diagonal extraction is one thin 2D DMA per w-parity (SP + Act
queues). A GpSimd add folds the 2x2 pool and an SP-queue DMA writes
the row out in bf16 (host upconverts during the assembly transpose).

DMA cost on trn2 is (free-dim bytes x 0.39ns) charged to the issuing
engine, so transfers are kept partition-fat/free-thin and spread over
the SP, Act and Pool queues to stay under the PE's 4.65us/row-pair.
Stages are software-pipelined with a skew (extract rp-1, add rp-2,
out rp-3) so no engine queue head-of-line blocks, and the last
row-pair's tail is dx-chunked so its write->extract->add->out chain
overlaps. Scale 1/(4*C) is folded into x1 on the host.
"""

import os

import numpy as np

import concourse.bacc as bacc
import concourse.bass as bass
import concourse.mybir as mybir
import concourse.tile as tile
from concourse.ap import AP
from concourse.bass import MemorySpace
from concourse.bass_utils import run_bass_kernel_spmd

MD = 20
K = 41
CC = K * K            # 1681
B, C, H, W = 4, 128, 96, 96
OH, OW = 48, 48
WP = W + 2 * MD       # 136
HH = 48               # full-res rows per core
NOH = 24              # output row-pairs per core
ROWS = HH + 2 * MD    # 88 x2p rows needed per core (h_local+dy <= 47+40)
GFREE = WP * K        # 5576 = per-partition Gram elems in [col, dy] layout

F32 = mybir.dt.float32
F32R = mybir.dt.float32r
BF16 = mybir.dt.bfloat16

LAST_EXEC_NS = None
_CACHED = None


def _build_nc():
    nc = bacc.Bacc("TRN2", target_bir_lowering=False)
    x1d = nc.dram_tensor("x1h", [C, HH * W], F32R, kind="ExternalInput")
    x2d = nc.dram_tensor("x2p", [C, ROWS * WP], F32R, kind="ExternalInput")
    outd = nc.dram_tensor("out", [NOH * OW, CC], BF16, kind="ExternalOutput")

    with tile.TileContext(nc) as tc:
        with (
            tc.tile_pool(name="inp", bufs=1) as inp_pool,
            tc.tile_pool(name="gsb", bufs=2) as gs_pool,
            tc.tile_pool(name="dd", bufs=2) as d_pool,
            tc.tile_pool(name="st", bufs=2) as s_pool,
            tc.tile_pool(name="ps", bufs=4, space=MemorySpace.PSUM) as psum_pool,
            tc.tile_pool(name="dr", bufs=2, space=MemorySpace.DRAM) as dram_pool,
        ):
            A = inp_pool.tile([C, HH * W], F32R)
            Bt = inp_pool.tile([C, ROWS * WP], F32R)
            # TRN2 ldweights encodes only ONE semaphore wait, so matmuls must
            # only ever depend on a single sem. Funnel input readiness through
            # the DVE counter (which later matmuls inherit transitively via
            # PSUM-eviction waits): DMA into staging tiles (SP/Act queues in
            # parallel), DVE-copy into A/Bt.
            # order loads so rp0's needs (x1 rows 0-11, x2 rows 0-43) are
            # staged first; DMAs alternate SP/Act queues
            loads = [("a", 0, 12), ("b", 0, 4), ("b", 4, 8), ("b", 12, 8),
                     ("b", 20, 8), ("b", 28, 8), ("b", 36, 8), ("a", 12, 12),
                     ("b", 44, 8), ("a", 24, 12), ("b", 52, 8), ("b", 60, 8),
                     ("a", 36, 12), ("b", 68, 8), ("b", 76, 8), ("b", 84, 4)]
            if os.environ.get("CORR_DIRECT_LOADS", "1") == "1":
                # DMA straight into A/Bt: matmuls then wait on two DMA-queue
                # sems plus the evictor sem (legalized by tile as standalone
                # PE waits).
                for i, (which, r0, n) in enumerate(loads):
                    eng = (nc.sync, nc.scalar, nc.gpsimd)[i % 3]
                    if which == "a":
                        eng.dma_start(A[:, r0 * W:(r0 + n) * W],
                                      x1d[:, r0 * W:(r0 + n) * W])
                    else:
                        eng.dma_start(Bt[:, r0 * WP:(r0 + n) * WP],
                                      x2d[:, r0 * WP:(r0 + n) * WP])
            else:
                with tc.tile_pool(name="stg", bufs=4) as stage_pool:
                    for i, (which, r0) in enumerate(loads):
                        eng = nc.sync if i % 2 == 0 else nc.scalar
                        if which == "a":
                            stg = stage_pool.tile([C, 12 * W], F32R, tag="stga")
                            eng.dma_start(stg[:], x1d[:, r0 * W:(r0 + 12) * W])
                            nc.vector.tensor_copy(A[:, r0 * W:(r0 + 12) * W], stg[:])
                        else:
                            stg = stage_pool.tile([C, 8 * WP], F32R, tag="stgb")
                            eng.dma_start(stg[:], x2d[:, r0 * WP:(r0 + 8) * WP])
                            nc.vector.tensor_copy(Bt[:, r0 * WP:(r0 + 8) * WP], stg[:])

            # Per-iteration stages are software-pipelined so no engine queue
            # ever head-of-line blocks on an unmet dependency: body rp issues
            # extraction for rp-1, pool-add for rp-2 and the output DMA for
            # rp-3 (their inputs completed in earlier iterations), then the
            # current Gram write last (it waits on this body's evictions).
            gbs, des, dos, ss = {}, {}, {}, {}

            def extract(r, dx0=0, dx1=K, engs=None):
                # diagonal extraction for row-pair r: element (u, dx, dy) of
                # parity p sits at flat (2u+p)*5576 + (2u+p+dx)*41 + dy =
                # 5617p + 11234u + dx*41 + dy; the (dx, dy) block is one
                # contiguous 1681-elem (3362B) run per u -> one thin 2D DMA
                # per parity (SP+Act queues).
                gt = gbs[r][:].tensor
                if r not in des:
                    des[r] = d_pool.tile([OW, CC], BF16, tag="de", name=f"de{r}")
                    dos[r] = d_pool.tile([OW, CC], BF16, tag="do", name=f"do{r}")
                shear = [[2 * (GFREE + K), OW], [K, dx1 - dx0], [1, K]]
                eeng, oeng = engs or (nc.sync, nc.scalar)
                eeng.dma_start(des[r][:, dx0 * K:dx1 * K],
                               AP(gt, dx0 * K, shear))
                oeng.dma_start(dos[r][:, dx0 * K:dx1 * K],
                               AP(gt, GFREE + K + dx0 * K, shear))

            def pool_add(r, dx0=0, dx1=K, eng=None):  # 2x2-pool finish
                if r not in ss:
                    ss[r] = s_pool.tile([OW, CC], BF16, tag="s", name=f"s{r}")
                sl = slice(dx0 * K, dx1 * K)
                (eng or nc.gpsimd).tensor_add(
                    ss[r][:, sl], des[r][:, sl], dos[r][:, sl])

            def out_dma(r, dx0=0, dx1=K, eng=None):
                (eng or nc.sync).dma_start(
                    outd[r * OW:(r + 1) * OW, dx0 * K:dx1 * K],
                    ss[r][:, dx0 * K:dx1 * K])

            for rp in range(NOH):
                h0 = 2 * rp
                a0 = A[:, h0 * W:(h0 + 1) * W]
                a1 = A[:, (h0 + 1) * W:(h0 + 2) * W]
                Gsb = gs_pool.tile([96, WP, K], BF16, tag="gsb")

                def mm(ps, j, h_ap, h_off, dy0, ncols):
                    nc.tensor.matmul(
                        ps[:, j, :ncols],
                        h_ap,
                        Bt[:, (h0 + h_off + dy0) * WP:
                              (h0 + h_off + dy0) * WP + ncols],
                        start=(h_off == 0), stop=(h_off == 1),
                    )

                # skewed stages first: all their deps are already complete
                if rp >= 1:
                    engs = (nc.sync, nc.sync) if rp == NOH - 1 else None
                    extract(rp - 1, engs=engs)
                if rp >= 2:
                    pool_add(rp - 2)
                if rp >= 3:
                    out_dma(rp - 3)

                # 7 psum tiles of 2 banks each (2 dy-triples), bufs=4 so the
                # PE runs up to 4 tiles ahead of the evictions, which
                # alternate DVE/Act per tile to split the copy load.
                for t in range(7):
                    ps = psum_pool.tile([96, 2, 512], F32, tag="ps")
                    ncols_b1 = 3 * WP if t < 6 else 2 * WP
                    for hoff, hap in ((0, a0), (1, a1)):
                        mm(ps, 0, hap, hoff, 6 * t, 3 * WP)
                        mm(ps, 1, hap, hoff, 6 * t + 3, ncols_b1)
                    ev = nc.vector if t % 2 == 0 else nc.scalar
                    evf = ev.tensor_copy if t % 2 == 0 else ev.copy
                    if t < 6:
                        evf(Gsb[:, :, 6 * t:6 * t + 6].transpose([0, 2, 1]),
                            ps[:, :, :3 * WP])
                    elif rp < NOH - 1:
                        evf(Gsb[:, :, 36:39].transpose([0, 2, 1]),
                            ps[:, 0, :3 * WP])
                        evf(Gsb[:, :, 39:41].transpose([0, 2, 1]),
                            ps[:, 1, :2 * WP])
                    else:
                        nc.vector.tensor_copy(
                            Gsb[:, :, 36:39].transpose([0, 2, 1]),
                            ps[:, 0, :3 * WP])
                        nc.scalar.copy(
                            Gsb[:, :, 39:41].transpose([0, 2, 1]),
                            ps[:, 1, :2 * WP])

                # Gram (bf16, [w, col*41+dy]) -> DRAM, split SP/Pool queues;
                # issued last: it waits on this body's evictions. The final
                # body splits 3 ways so the drain starts sooner.
                Gb = dram_pool.tile([96, GFREE], BF16, tag="gb")
                gbs[rp] = Gb
                if rp < NOH - 1:
                    nc.sync.dma_start(Gb[:, :45 * K], Gsb[:, :45, :])
                    nc.gpsimd.dma_start(Gb[:, 45 * K:], Gsb[:, 45:, :])
                else:
                    # keep SP free of the last write so the drain's
                    # extractions are not head-of-line blocked behind it
                    nc.scalar.dma_start(Gb[:, :45 * K], Gsb[:, :45, :])
                    nc.gpsimd.dma_start(Gb[:, 45 * K:90 * K], Gsb[:, 45:90, :])
                    nc.scalar.dma_start(Gb[:, 90 * K:], Gsb[:, 90:, :])

            # drain the pipeline; the last row-pair's tail stages are chunked
            # along dx so extract/add/out overlap instead of serializing.
            L = NOH - 1
            CH = [(0, 21), (21, 41)]
            out_dma(L - 2)
            pool_add(L - 1)
            for dx0, dx1 in CH:
                extract(L, dx0, dx1)
            out_dma(L - 1, eng=nc.scalar)
            for i, (dx0, dx1) in enumerate(CH):
                pool_add(L, dx0, dx1)
                out_dma(L, dx0, dx1, eng=nc.sync if i % 2 == 0 else nc.scalar)
    nc.compile()
    return nc


def kernel(x1: np.ndarray, x2: np.ndarray) -> np.ndarray:
    global LAST_EXEC_NS, _CACHED
    x1 = np.ascontiguousarray(np.asarray(x1, dtype=np.float32)) * np.float32(1.0 / (4 * C))
    x2 = np.asarray(x2, dtype=np.float32)
    x2p = np.zeros((B, C, 2 * MD + H, WP), dtype=np.float32)
    x2p[:, :, MD:MD + H, MD:MD + W] = x2

    if _CACHED is None:
        _CACHED = _build_nc()
    nc = _CACHED

    in_maps = []
    for core in range(8):
        b, half = core // 2, core % 2
        a = np.ascontiguousarray(
            x1[b, :, half * HH:(half + 1) * HH, :].reshape(C, HH * W))
        x2s = np.ascontiguousarray(
            x2p[b, :, half * HH:half * HH + ROWS, :].reshape(C, ROWS * WP))
        in_maps.append({"x1h": a, "x2p": x2s})

    try:
        res = run_bass_kernel_spmd(
            nc, in_maps, core_ids=list(range(8)),
            trace=os.environ.get("CORR_TRACE") == "1",
        )
    except ImportError:
        res = run_bass_kernel_spmd(nc, in_maps, core_ids=list(range(8)))
    LAST_EXEC_NS = res.exec_time_ns

    out = np.empty((B, CC, OH, OW), dtype=np.float32)
    for core in range(8):
        b, half = core // 2, core % 2
        r = np.asarray(res.results[core]["out"]).reshape(NOH, OW, CC)
        out[b, :, half * NOH:(half + 1) * NOH, :] = r.transpose(2, 0, 1)
    return out



# revision 89
# speedup vs baseline: 1.2285x; 1.2285x over previous
"""FlowNet-C correlation layer (MAX_DISP=20, STRIDE=2) on 8 trn2 cores.

Strategy: shard by (batch b, output-row half). Core k handles b=k//2,
output rows [24*(k%2), 24*(k%2)+24). Contraction over C=128 runs on the
TensorEngine as banded-Gram matmuls in bf16: per output row pair
(h0,h0+1) and 5-dy group g, psum[96, 5*96] += x1_row[128,96]^T @
x2rows[128,480] with the h-pair pooled via PSUM accumulation. Moving
columns span only the 96 REAL x2 columns (no horizontal zero padding),
so PE cost is 2*41*96 = 7872 cycles/row-pair instead of 11152.

PSUM is evicted to a [w1, w2, dy] bf16 SBUF tile (dy innermost),
split across DVE/Act/Pool so no engine exceeds the PE's 3.28us/rp,
then one SP DMA writes it into the middle of a padded [96, 136*41]
DRAM row (the 20-column pads are zeroed once per buffer at startup).
In that layout the 1681 wanted (dx, dy) elements for one output pixel
are a single contiguous 3362B run, so diagonal extraction is one thin
2D DMA per w-parity (SP + Act queues). A DVE add (bf16 2x mode) folds
the 2x2 pool and an SP DMA writes the row out in bf16.

Stages are software-pipelined with a skew (extract rp-1, add rp-2,
out rp-3); the last row-pair's tail is dx-chunked so its
write->extract->add->out chain overlaps. Scale 1/(4*C) is folded into
x1 on the host; inputs are pre-converted to bf16 host-side.
"""

import os

import numpy as np
import ml_dtypes

import concourse.bacc as bacc
import concourse.bass as bass
import concourse.mybir as mybir
import concourse.tile as tile
from concourse.ap import AP
from concourse.bass import MemorySpace
from concourse.bass_utils import run_bass_kernel_spmd

MD = 20
K = 41
CC = K * K            # 1681
B, C, H, W = 4, 128, 96, 96
OH, OW = 48, 48
HH = 48               # full-res rows per core
NOH = 24              # output row-pairs per core
XROWS = 78            # x2 rows per core: 10 host-zeroed + 68 real
                      # (buffer row = local padded row - 10; both halves are
                      # fed "top-half" structure -- half 1 is flipped on the
                      # host -- so padded rows 0-9 are all-zero and never
                      # referenced: groups needing them are skipped)
GW = 96               # gram moving width (real columns only)
GFREE = (W + 2 * MD) * K  # 5576 = padded per-partition gram row in DRAM
PADL = MD * K         # 820 = left pad cols in the DRAM gram row

F32 = mybir.dt.float32
BF16 = mybir.dt.bfloat16

LAST_EXEC_NS = None
_CACHED = None


def _build_nc():
    nc = bacc.Bacc("TRN2", target_bir_lowering=False)
    x1d = nc.dram_tensor("x1h", [C, HH * W], BF16, kind="ExternalInput")
    x2d = nc.dram_tensor("x2p", [C, XROWS * W], BF16, kind="ExternalInput")
    outd = nc.dram_tensor("out", [NOH * OW, CC], BF16, kind="ExternalOutput")

    with tile.TileContext(nc) as tc:
        with (
            tc.tile_pool(name="inp", bufs=1) as inp_pool,
            tc.tile_pool(name="gsb", bufs=3) as gs_pool,
            tc.tile_pool(name="dd", bufs=3) as d_pool,
            tc.tile_pool(name="st", bufs=3) as s_pool,
            tc.tile_pool(name="ps", bufs=3, space=MemorySpace.PSUM) as psum_pool,
            tc.tile_pool(name="pss", bufs=2, space=MemorySpace.PSUM) as psums_pool,
            tc.tile_pool(name="dr", bufs=3, space=MemorySpace.DRAM) as dram_pool,
            tc.tile_pool(name="drs", bufs=2, space=MemorySpace.DRAM) as drams_pool,
        ):
            A = inp_pool.tile([C, HH * W], BF16)
            Bt = inp_pool.tile([C, XROWS * W], BF16)
            Z = inp_pool.tile([96, 4 * PADL], BF16)
            nc.vector.memset(Z[:], 0.0)

            # loads are hand-scheduled: rp0's needs (x1 rows 0-1, x2 rows
            # 0-31) land first split over the three queues; the bulk is
            # injected into early loop bodies after those bodies' critical
            # DMAs so the queues never starve the per-rp pipeline.
            def load(which, r0, n, eng):
                if which == "a":
                    eng.dma_start(A[:, r0 * W:(r0 + n) * W],
                                  x1d[:, r0 * W:(r0 + n) * W])
                else:
                    eng.dma_start(Bt[:, r0 * W:(r0 + n) * W],
                                  x2d[:, r0 * W:(r0 + n) * W])

            load("a", 0, 2, nc.sync)
            load("b", 0, 11, nc.sync)
            load("b", 11, 11, nc.scalar)
            load("b", 22, 12, nc.gpsimd)
            load("a", 2, 8, nc.sync)
            # bulk loads injected into early bodies below:
            bulk_loads = {0: [("b", 34, 22, "gpsimd")],
                          1: [("a", 10, 6, "scalar"),
                              ("b", 56, 22, "gpsimd")],
                          2: [("a", 16, 32, "sync")]}

            # Per-iteration stages are software-pipelined so no engine queue
            # head-of-line blocks: body rp issues extraction for rp-1,
            # pool-add for rp-2 and the output DMA for rp-3 (their inputs
            # completed in earlier iterations), then this rp's Gram write
            # last (it waits on this body's evictions).
            # gram/extract/add are per row-pair; only the OUT stage is
            # merged over pairs of row-pairs (one DMA per 2 rps) to halve
            # its fixed overhead -- that's what lets Act fit its budget.
            gbs, gbs_s, des, ss = {}, {}, {}, {}

            G2 = 2 * GFREE

            def extract(pp, r0=0, r1=2, eng=None):
                # diagonal extraction for pair pp (row-pairs 2pp, 2pp+1
                # stacked in one DRAM tile): element (u, p, r, dx, dy) sits
                # at flat (2u+p)*2*5576 + r*5576 + ((2u+p)+dx)*41 + dy; the
                # (dx, dy) block is one contiguous 1681-elem (3362B) run
                # per (u, p, r) -> one thin 4D DMA per pair.
                gt = gbs[pp][:].tensor
                if pp not in des:
                    des[pp] = d_pool.tile([OW, 2, 2, CC], BF16, tag="de",
                                          name=f"de{pp}")
                shear = [[2 * (G2 + K), OW], [G2 + K, 2],
                         [GFREE, r1 - r0], [1, CC]]
                (eng or nc.gpsimd).dma_start(
                    des[pp][:, :, r0:r1, :], AP(gt, r0 * GFREE, shear))

            gsbs = {}

            # TWO persistent DRAM pair tiles, reused alternately by all
            # pairs (pool tiles get fresh DRAM per generation, so the pad
            # zeroing below would not survive re-allocation). Reusing the
            # same tile objects also gives exact write-after-read ordering
            # from tile's whole-tensor tracking.
            gb2 = [dram_pool.tile([96, G2], BF16, tag="gb", name="gbA"),
                   dram_pool.tile([96, G2], BF16, tag="gb", name="gbB"),
                   dram_pool.tile([96, G2], BF16, tag="gb", name="gbC")]
            for i, (gb, eng) in enumerate(
                    zip(gb2, (nc.scalar, nc.gpsimd, nc.scalar))):
                gt = gb[:].tensor
                for hh in (0, 1):  # 3-dim APs: one per pair half
                    eng.dma_start(
                        AP(gt, hh * GFREE,
                           [[G2, 96], [GFREE - PADL, 2], [1, PADL]]),
                        Z[:, :2 * PADL].rearrange("p (a b) -> p a b", a=2))

            def gram_write(r):
                # Gram (bf16, [w1, w2*41+dy]) -> middle of rp r's half of
                # the pair's padded DRAM tile, one SP DMA; pad columns were
                # zeroed once at startup, never rewritten. The last two rps
                # get their own single tiles.
                if r >= 22:
                    nc.sync.dma_start(
                        gbs_s[r][:, PADL:GFREE - PADL], gsbs[r][:])
                    return
                gbs[r // 2] = Gb = gb2[(r // 2) % 3]
                half = r % 2
                nc.sync.dma_start(
                    Gb[:, half * GFREE + PADL:half * GFREE + GFREE - PADL],
                    gsbs[r][:])

            def extract_single(r, eng):
                # last two row-pairs live in their own single-rp DRAM tiles
                # so each drain extraction depends only on its own gram
                # write (tile deps are whole-tile for raw APs).
                gt = gbs_s[r][:].tensor
                pp = r // 2
                if pp not in des:
                    des[pp] = d_pool.tile([OW, 2, 2, CC], BF16, tag="de",
                                          name=f"de{pp}")
                shear = [[2 * (GFREE + K), OW], [GFREE + K, 2], [1, CC]]
                eng.dma_start(des[pp][:, :, r % 2, :], AP(gt, 0, shear))

            def pool_add(r, eng=None):  # 2x2-pool finish
                q = r // 4
                if q not in ss:
                    ss[q] = s_pool.tile([OW, 4, CC], BF16, tag="s",
                                        name=f"s{q}")
                (eng or nc.vector).tensor_add(
                    ss[q][:, r % 4, :], des[r // 2][:, 0, r % 2, :],
                    des[r // 2][:, 1, r % 2, :])

            def out_dma(q, r0=0, r1=4, eng=None):
                # one DMA per 4 row-pairs; dst row = (4*q + r)*48 + u
                ot = outd[:].tensor
                (eng or nc.scalar).dma_start(
                    AP(ot, (4 * q + r0) * OW * CC,
                       [[CC, OW], [OW * CC, r1 - r0], [1, CC]]),
                    ss[q][:, r0:r1, :])

            # dy-groups (10,10,10,10,1): each 10-group is a [96, 2, 512]
            # 2-bank PSUM tile (a matmul output must stay inside ONE 2KB
            # bank -> 5 dys per matmul), evicted by a single 4D-AP copy.
            # Eviction engines DVE/Act alternating (GPSIMD cannot access
            # PSUM); keeps every engine under the PE's ~3.3us/row-pair.
            ev_engs = [nc.vector.tensor_copy, nc.scalar.copy,
                       nc.vector.tensor_copy, nc.scalar.copy,
                       nc.scalar.copy]

            for rp in range(NOH):
                h0 = 2 * rp
                a0 = A[:, h0 * W:(h0 + 1) * W]
                a1 = A[:, (h0 + 1) * W:(h0 + 2) * W]
                Gsb = gs_pool.tile([96, GW, K], BF16, tag="gsb",
                                   name=f"gsb{rp}")
                gsbs[rp] = Gsb
                if rp < 5:
                    # rps 0-4 skip group g0, so its dy<10 region would
                    # otherwise reach DRAM uninitialized; zero it (also
                    # makes those output channels exactly right already)
                    nc.vector.memset(Gsb[:, :, 0:10], 0.0)

                # 4 groups of 10 dys + single-dy tail; h-pair pooled via
                # PSUM accumulate (start on hoff 0, stop on hoff 1).
                # Buffer row for (h, dy) is h + dy - 10; rows < 10 of group
                # g0 only exist for rp >= 5, earlier rps skip g0 entirely
                # (channels provably zero, fixed up after the pool-add).
                g_lo = 0 if rp >= 5 else 1
                for g in range(g_lo, 4):
                    d0 = 10 * g
                    ps = psum_pool.tile([96, 2, 512], F32, tag="ps",
                                        name=f"ps{rp}_{g}")
                    for hoff, hap in ((0, a0), (1, a1)):
                        for j in (0, 1):
                            r0 = h0 + hoff + d0 + 5 * j - 10
                            nc.tensor.matmul(
                                ps[:, j, :5 * GW],
                                hap,
                                Bt[:, r0 * W:r0 * W + 5 * GW],
                                start=(hoff == 0), stop=(hoff == 1),
                            )
                    ev_engs[g](
                        Gsb[:, :, d0:d0 + 10].rearrange(
                            "p w (j d) -> p j d w", j=2),
                        ps[:, :, :5 * GW].rearrange(
                            "p j (d w) -> p j d w", w=GW))
                pss = psums_pool.tile([96, GW], F32, tag="ps5",
                                      name=f"pss{rp}")
                for hoff, hap in ((0, a0), (1, a1)):
                    r0 = h0 + hoff + 30
                    nc.tensor.matmul(
                        pss[:], hap, Bt[:, r0 * W:r0 * W + GW],
                        start=(hoff == 0), stop=(hoff == 1),
                    )
                ev_engs[4](Gsb[:, :, 40], pss[:])

                # skewed stages AFTER this body's evictions so they never
                # head-of-line block the eviction dispatches; every DMA's
                # deps are complete at dispatch (gram is skewed one body,
                # extract two, add/out deeper), so each queue runs
                # back-to-back at transfer rate instead of paying the
                # ~1.3us DGE setup latency per DMA.
                if rp >= 1:
                    gram_write(rp - 1)
                if rp % 2 == 0 and rp >= 4:
                    extract((rp - 4) // 2,
                            eng=getattr(nc, os.environ.get("CORR_XE", "gpsimd")))
                if rp >= 5:
                    pool_add(rp - 5)
                om = os.environ.get("CORR_OM", "split2")
                if om in ("pool", "act"):
                    if rp % 4 == 1 and rp >= 9:
                        out_dma((rp - 9) // 4,
                                eng=nc.gpsimd if om == "pool" else nc.scalar)
                else:  # staggered halves on Act and Pool
                    if rp % 4 == 1 and rp >= 9:
                        out_dma((rp - 9) // 4, 0, 2, eng=nc.scalar)
                    if rp % 4 == 3 and rp >= 11:
                        out_dma((rp - 11) // 4, 2, 4, eng=nc.gpsimd)

                # bulk input loads, injected after the early bodies'
                # critical-path work so queues stay responsive
                for which, r0, n, engname in bulk_loads.get(rp, []):
                    load(which, r0, n, getattr(nc, engname))

                # single-rp gram tiles for the last two rps (created and
                # pad-zeroed mid-stream) so the drain chains decouple
                if rp in (4, 6):
                    r_late = 22 + (rp - 4) // 2
                    gbs_s[r_late] = drams_pool.tile(
                        [96, GFREE], BF16, tag="gbs", name=f"gbs{r_late}")
                    nc.scalar.dma_start(
                        AP(gbs_s[r_late][:].tensor, 0,
                           [[GFREE, 96], [GFREE - PADL, 2], [1, PADL]]),
                        Z[:, :2 * PADL].rearrange("p (a b) -> p a b", a=2))

            # drain the pipeline. The critical chain is
            # evict(23) -> gram(23) [SP] -> extract(23) [Act] -> add(23)
            # [DVE] -> out [SP]; bulk leftovers (extract(10), out(4)) go to
            # Pool so they never block that chain's engines.
            gram_write(23)                       # SP, behind gram(22) only
            extract(10, eng=nc.gpsimd)           # rps 20, 21
            extract_single(22, eng=nc.scalar)    # gram(22) done
            extract_single(23, eng=nc.scalar)    # waits gram(23)
            pool_add(19)
            out_dma(4, eng=nc.gpsimd)            # rps 16-19
            pool_add(20)
            pool_add(21)
            pool_add(22)
            pool_add(23)
            out_dma(5, 0, 2, eng=nc.gpsimd)      # rps 20, 21
            out_dma(5, 2, 3, eng=nc.scalar)      # rp 22
            out_dma(5, 3, 4, eng=nc.sync)        # rp 23
    nc.compile()
    return nc


def kernel(x1: np.ndarray, x2: np.ndarray) -> np.ndarray:
    global LAST_EXEC_NS, _CACHED
    x1 = np.asarray(x1, dtype=np.float32) * np.float32(1.0 / (4 * C))
    x1 = x1.astype(ml_dtypes.bfloat16)
    x2 = np.asarray(x2, dtype=np.float32).astype(ml_dtypes.bfloat16)
    # vertical zero-pad only; matmuls never touch horizontal pads.
    # Half-1 cores get vertically FLIPPED inputs so every core sees the
    # same "top-half" structure (zero pad rows at small local indices);
    # the flip negates dy, undone during host reassembly.
    x2pv = np.zeros((B, C, H + 2 * MD, W), dtype=ml_dtypes.bfloat16)
    x2pv[:, :, MD:MD + H, :] = x2

    if _CACHED is None:
        _CACHED = _build_nc()
    nc = _CACHED

    in_maps = []
    for core in range(8):
        b, half = core // 2, core % 2
        if half == 0:
            a = x1[b, :, 0:HH, :]
            x2s = x2pv[b, :, 10:10 + XROWS, :]
        else:
            a = x1[b, :, :HH - 1:-1, :]               # rows 95..48
            x2s = x2pv[b, :, 125:125 - XROWS:-1, :]   # padded 125..48
        in_maps.append({
            "x1h": np.ascontiguousarray(a.reshape(C, HH * W)),
            "x2p": np.ascontiguousarray(x2s.reshape(C, XROWS * W)),
        })

    try:
        res = run_bass_kernel_spmd(
            nc, in_maps, core_ids=list(range(8)),
            trace=os.environ.get("CORR_TRACE") == "1",
        )
    except ImportError:
        res = run_bass_kernel_spmd(nc, in_maps, core_ids=list(range(8)))
    LAST_EXEC_NS = res.exec_time_ns

    out = np.empty((B, CC, OH, OW), dtype=np.float32)
    for core in range(8):
        b, half = core // 2, core % 2
        r = np.asarray(res.results[core]["out"]).reshape(NOH, OW, CC)
        if half == 0:
            out[b, :, 0:NOH, :] = r.transpose(2, 0, 1)
        else:
            # local rp -> global pooled row 47-rp; local dy j -> 40-j
            rr = r.reshape(NOH, OW, K, K)[::-1, :, :, ::-1]
            out[b, :, NOH:2 * NOH, :] = (
                rr.reshape(NOH, OW, CC).transpose(2, 0, 1))
    return out


# revision 99
# speedup vs baseline: 1.2808x; 1.0426x over previous
"""FlowNet-C correlation layer (MAX_DISP=20, STRIDE=2) on 8 trn2 cores.

Strategy: shard by (batch b, output-row half). Core k handles b=k//2,
output rows [24*(k%2), 24*(k%2)+24). Contraction over C=128 runs on the
TensorEngine as banded-Gram matmuls in bf16: per output row pair
(h0,h0+1) and 5-dy group g, psum[96, 5*96] += x1_row[128,96]^T @
x2rows[128,480] with the h-pair pooled via PSUM accumulation. Moving
columns span only the 96 REAL x2 columns (no horizontal zero padding),
so PE cost is 2*41*96 = 7872 cycles/row-pair instead of 11152.

PSUM is evicted to a [w1, w2, dy] bf16 SBUF tile (dy innermost),
split across DVE/Act/Pool so no engine exceeds the PE's 3.28us/rp,
then one SP DMA writes it into the middle of a padded [96, 136*41]
DRAM row (the 20-column pads are zeroed once per buffer at startup).
In that layout the 1681 wanted (dx, dy) elements for one output pixel
are a single contiguous 3362B run, so diagonal extraction is one thin
2D DMA per w-parity (SP + Act queues). A DVE add (bf16 2x mode) folds
the 2x2 pool and an SP DMA writes the row out in bf16.

Stages are software-pipelined with a skew (extract rp-1, add rp-2,
out rp-3); the last row-pair's tail is dx-chunked so its
write->extract->add->out chain overlaps. Scale 1/(4*C) is folded into
x1 on the host; inputs are pre-converted to bf16 host-side.
"""

import os

import numpy as np
import ml_dtypes

import concourse.bacc as bacc
import concourse.bass as bass
import concourse.mybir as mybir
import concourse.tile as tile
from concourse.ap import AP
from concourse.bass import MemorySpace
from concourse.bass_utils import run_bass_kernel_spmd

MD = 20
K = 41
CC = K * K            # 1681
B, C, H, W = 4, 128, 96, 96
OH, OW = 48, 48
HH = 48               # full-res rows per core
NOH = 24              # output row-pairs per core
XROWS = 78            # x2 rows per core: 10 host-zeroed + 68 real
                      # (buffer row = local padded row - 10; both halves are
                      # fed "top-half" structure -- half 1 is flipped on the
                      # host -- so padded rows 0-9 are all-zero and never
                      # referenced: groups needing them are skipped)
GW = 96               # gram moving width (real columns only)
GFREE = (W + 2 * MD) * K  # 5576 = padded per-partition gram row in DRAM
PADL = MD * K         # 820 = left pad cols in the DRAM gram row

F32 = mybir.dt.float32
BF16 = mybir.dt.bfloat16

LAST_EXEC_NS = None
_CACHED = None


def _build_nc():
    nc = bacc.Bacc("TRN2", target_bir_lowering=False)
    x1d = nc.dram_tensor("x1h", [C, HH * W], BF16, kind="ExternalInput")
    x2d = nc.dram_tensor("x2p", [C, XROWS * W], BF16, kind="ExternalInput")
    outd = nc.dram_tensor("out", [NOH * OW, CC], BF16, kind="ExternalOutput")

    with tile.TileContext(nc) as tc:
        with (
            tc.tile_pool(name="inp", bufs=1) as inp_pool,
            tc.tile_pool(name="gsb", bufs=3) as gs_pool,
            tc.tile_pool(name="dd", bufs=3) as d_pool,
            tc.tile_pool(name="st", bufs=3) as s_pool,
            tc.tile_pool(name="ps", bufs=3, space=MemorySpace.PSUM) as psum_pool,
            tc.tile_pool(name="pss", bufs=2, space=MemorySpace.PSUM) as psums_pool,
            tc.tile_pool(name="dr", bufs=3, space=MemorySpace.DRAM) as dram_pool,
            tc.tile_pool(name="drs", bufs=2, space=MemorySpace.DRAM) as drams_pool,
        ):
            A = inp_pool.tile([C, HH * W], BF16)
            Bt = inp_pool.tile([C, XROWS * W], BF16)
            Z = inp_pool.tile([96, 4 * PADL], BF16)
            nc.vector.memset(Z[:], 0.0)

            # loads are hand-scheduled: rp0's needs (x1 rows 0-1, x2 rows
            # 0-31) land first split over the three queues; the bulk is
            # injected into early loop bodies after those bodies' critical
            # DMAs so the queues never starve the per-rp pipeline.
            def load(which, r0, n, eng):
                if which == "a":
                    eng.dma_start(A[:, r0 * W:(r0 + n) * W],
                                  x1d[:, r0 * W:(r0 + n) * W])
                else:
                    eng.dma_start(Bt[:, r0 * W:(r0 + n) * W],
                                  x2d[:, r0 * W:(r0 + n) * W])

            load("a", 0, 2, nc.sync)
            load("b", 0, 11, nc.sync)
            load("b", 11, 11, nc.scalar)
            load("b", 22, 12, nc.gpsimd)
            load("a", 2, 8, nc.sync)
            # bulk loads injected into early bodies below:
            bulk_loads = {0: [("b", 34, 22, "gpsimd")],
                          1: [("a", 10, 6, "scalar"),
                              ("b", 56, 22, "gpsimd")],
                          2: [("a", 16, 32, "sync")]}

            # Per-iteration stages are software-pipelined so no engine queue
            # head-of-line blocks: body rp issues extraction for rp-1,
            # pool-add for rp-2 and the output DMA for rp-3 (their inputs
            # completed in earlier iterations), then this rp's Gram write
            # last (it waits on this body's evictions).
            # gram/extract/add are per row-pair; only the OUT stage is
            # merged over pairs of row-pairs (one DMA per 2 rps) to halve
            # its fixed overhead -- that's what lets Act fit its budget.
            gbs, gbs_s, des, ss = {}, {}, {}, {}

            G2 = 2 * GFREE

            def extract(pp, r0=0, r1=2, eng=None):
                # diagonal extraction for pair pp (row-pairs 2pp, 2pp+1
                # stacked in one DRAM tile): element (u, p, r, dx, dy) sits
                # at flat (2u+p)*2*5576 + r*5576 + ((2u+p)+dx)*41 + dy; the
                # (dx, dy) block is one contiguous 1681-elem (3362B) run
                # per (u, p, r) -> one thin 4D DMA per pair.
                gt = gbs[pp][:].tensor
                if pp not in des:
                    des[pp] = d_pool.tile([OW, 2, 2, CC], BF16, tag="de",
                                          name=f"de{pp}")
                shear = [[2 * (G2 + K), OW], [G2 + K, 2],
                         [GFREE, r1 - r0], [1, CC]]
                (eng or nc.gpsimd).dma_start(
                    des[pp][:, :, r0:r1, :], AP(gt, r0 * GFREE, shear))

            gsbs = {}

            # TWO persistent DRAM pair tiles, reused alternately by all
            # pairs (pool tiles get fresh DRAM per generation, so the pad
            # zeroing below would not survive re-allocation). Reusing the
            # same tile objects also gives exact write-after-read ordering
            # from tile's whole-tensor tracking.
            gb2 = [dram_pool.tile([96, G2], BF16, tag="gb", name="gbA"),
                   dram_pool.tile([96, G2], BF16, tag="gb", name="gbB"),
                   dram_pool.tile([96, G2], BF16, tag="gb", name="gbC")]

            def zero_pads(i, eng):
                gt = gb2[i][:].tensor
                for hh in (0, 1):  # 3-dim APs: one per pair half
                    eng.dma_start(
                        AP(gt, hh * GFREE,
                           [[G2, 96], [GFREE - PADL, 2], [1, PADL]]),
                        Z[:, :2 * PADL].rearrange("p (a b) -> p a b", a=2))

            # slot i first read by extract(pair i) at body 2i+4; stagger
            # the zeroing across queues/bodies to keep startup responsive
            zero_pads(0, nc.sync)

            def gram_write(r):
                # Gram (bf16, [w1, w2*41+dy]) -> middle of rp r's half of
                # the pair's padded DRAM tile, one SP DMA; pad columns were
                # zeroed once at startup, never rewritten. The last two rps
                # get their own single tiles.
                if r >= 22:
                    nc.sync.dma_start(
                        gbs_s[r][:, PADL:GFREE - PADL], gsbs[r][:])
                    return
                gbs[r // 2] = Gb = gb2[(r // 2) % 3]
                half = r % 2
                nc.sync.dma_start(
                    Gb[:, half * GFREE + PADL:half * GFREE + GFREE - PADL],
                    gsbs[r][:])

            def extract_single(r, eng):
                # last two row-pairs live in their own single-rp DRAM tiles
                # so each drain extraction depends only on its own gram
                # write (tile deps are whole-tile for raw APs).
                gt = gbs_s[r][:].tensor
                pp = r // 2
                if pp not in des:
                    des[pp] = d_pool.tile([OW, 2, 2, CC], BF16, tag="de",
                                          name=f"de{pp}")
                shear = [[2 * (GFREE + K), OW], [GFREE + K, 2], [1, CC]]
                eng.dma_start(des[pp][:, :, r % 2, :], AP(gt, 0, shear))

            def pool_add(r, eng=None):  # 2x2-pool finish
                q = r // 4
                if q not in ss:
                    ss[q] = s_pool.tile([OW, 4, CC], BF16, tag="s",
                                        name=f"s{q}")
                (eng or nc.vector).tensor_add(
                    ss[q][:, r % 4, :], des[r // 2][:, 0, r % 2, :],
                    des[r // 2][:, 1, r % 2, :])

            def out_dma(q, r0=0, r1=4, eng=None):
                # one DMA per 4 row-pairs; dst row = (4*q + r)*48 + u
                ot = outd[:].tensor
                (eng or nc.scalar).dma_start(
                    AP(ot, (4 * q + r0) * OW * CC,
                       [[CC, OW], [OW * CC, r1 - r0], [1, CC]]),
                    ss[q][:, r0:r1, :])

            # dy-groups (10,10,10,10,1): each 10-group is a [96, 2, 512]
            # 2-bank PSUM tile (a matmul output must stay inside ONE 2KB
            # bank -> 5 dys per matmul), evicted by a single 4D-AP copy.
            # Eviction engines DVE/Act alternating (GPSIMD cannot access
            # PSUM); keeps every engine under the PE's ~3.3us/row-pair.
            ev_engs = [nc.vector.tensor_copy, nc.scalar.copy,
                       nc.vector.tensor_copy, nc.scalar.copy,
                       nc.scalar.copy]

            for rp in range(NOH):
                h0 = 2 * rp
                a0 = A[:, h0 * W:(h0 + 1) * W]
                a1 = A[:, (h0 + 1) * W:(h0 + 2) * W]
                Gsb = gs_pool.tile([96, GW, K], BF16, tag="gsb",
                                   name=f"gsb{rp}")
                gsbs[rp] = Gsb
                if rp < 5:
                    # rps 0-4 skip group g0, so its dy<10 region would
                    # otherwise reach DRAM uninitialized; zero it (also
                    # makes those output channels exactly right already)
                    meng = nc.vector if rp % 2 == 0 else nc.gpsimd
                    meng.memset(Gsb[:, :, 0:10], 0.0)

                # 4 groups of 10 dys + single-dy tail; h-pair pooled via
                # PSUM accumulate (start on hoff 0, stop on hoff 1).
                # Buffer row for (h, dy) is h + dy - 10; rows < 10 of group
                # g0 only exist for rp >= 5, earlier rps skip g0 entirely
                # (channels provably zero, fixed up after the pool-add).
                g_lo = 0 if rp >= 5 else 1
                for g in range(g_lo, 4):
                    d0 = 10 * g
                    ps = psum_pool.tile([96, 2, 512], F32, tag="ps",
                                        name=f"ps{rp}_{g}")
                    for hoff, hap in ((0, a0), (1, a1)):
                        for j in (0, 1):
                            r0 = h0 + hoff + d0 + 5 * j - 10
                            nc.tensor.matmul(
                                ps[:, j, :5 * GW],
                                hap,
                                Bt[:, r0 * W:r0 * W + 5 * GW],
                                start=(hoff == 0), stop=(hoff == 1),
                            )
                    ev_engs[g](
                        Gsb[:, :, d0:d0 + 10].rearrange(
                            "p w (j d) -> p j d w", j=2),
                        ps[:, :, :5 * GW].rearrange(
                            "p j (d w) -> p j d w", w=GW))
                pss = psums_pool.tile([96, GW], F32, tag="ps5",
                                      name=f"pss{rp}")
                for hoff, hap in ((0, a0), (1, a1)):
                    r0 = h0 + hoff + 30
                    nc.tensor.matmul(
                        pss[:], hap, Bt[:, r0 * W:r0 * W + GW],
                        start=(hoff == 0), stop=(hoff == 1),
                    )
                ev_engs[4](Gsb[:, :, 40], pss[:])

                # skewed stages AFTER this body's evictions so they never
                # head-of-line block the eviction dispatches; every DMA's
                # deps are complete at dispatch (gram is skewed one body,
                # extract two, add/out deeper), so each queue runs
                # back-to-back at transfer rate instead of paying the
                # ~1.3us DGE setup latency per DMA.
                if rp >= 1:
                    gram_write(rp - 1)
                if rp == 1:
                    zero_pads(1, nc.gpsimd)
                elif rp == 3:
                    zero_pads(2, nc.scalar)
                if rp % 2 == 0 and rp >= 4:
                    extract((rp - 4) // 2,
                            eng=getattr(nc, os.environ.get("CORR_XE", "gpsimd")))
                if rp >= 5:
                    pool_add(rp - 5)
                om = os.environ.get("CORR_OM", "split2")
                if om in ("pool", "act"):
                    if rp % 4 == 1 and rp >= 9:
                        out_dma((rp - 9) // 4,
                                eng=nc.gpsimd if om == "pool" else nc.scalar)
                else:  # staggered halves on Act and Pool
                    if rp % 4 == 1 and rp >= 9:
                        out_dma((rp - 9) // 4, 0, 2, eng=nc.scalar)
                    if rp % 4 == 3 and rp >= 11:
                        out_dma((rp - 11) // 4, 2, 4, eng=nc.gpsimd)

                # bulk input loads, injected after the early bodies'
                # critical-path work so queues stay responsive
                for which, r0, n, engname in bulk_loads.get(rp, []):
                    load(which, r0, n, getattr(nc, engname))

                # single-rp gram tiles for the last two rps (created and
                # pad-zeroed mid-stream) so the drain chains decouple
                if rp in (4, 6):
                    r_late = 22 + (rp - 4) // 2
                    gbs_s[r_late] = drams_pool.tile(
                        [96, GFREE], BF16, tag="gbs", name=f"gbs{r_late}")
                    nc.scalar.dma_start(
                        AP(gbs_s[r_late][:].tensor, 0,
                           [[GFREE, 96], [GFREE - PADL, 2], [1, PADL]]),
                        Z[:, :2 * PADL].rearrange("p (a b) -> p a b", a=2))

            # drain the pipeline. The critical chain is
            # evict(23) -> gram(23) [SP] -> extract(23) [Act] -> add(23)
            # [DVE] -> out [SP]; bulk leftovers (extract(10), out(4)) go to
            # Pool so they never block that chain's engines.
            gram_write(23)                       # SP, behind gram(22) only
            extract(10, eng=nc.gpsimd)           # rps 20, 21
            extract_single(22, eng=nc.scalar)    # gram(22) done
            extract_single(23, eng=nc.scalar)    # waits gram(23)
            pool_add(19)
            out_dma(4, eng=nc.gpsimd)            # rps 16-19
            pool_add(20)
            pool_add(21)
            pool_add(22)
            pool_add(23)
            out_dma(5, 0, 2, eng=nc.gpsimd)      # rps 20, 21
            out_dma(5, 2, 3, eng=nc.scalar)      # rp 22
            out_dma(5, 3, 4, eng=nc.sync)        # rp 23
    nc.compile()
    return nc


def kernel(x1: np.ndarray, x2: np.ndarray) -> np.ndarray:
    global LAST_EXEC_NS, _CACHED
    x1 = np.asarray(x1, dtype=np.float32) * np.float32(1.0 / (4 * C))
    x1 = x1.astype(ml_dtypes.bfloat16)
    x2 = np.asarray(x2, dtype=np.float32).astype(ml_dtypes.bfloat16)
    # vertical zero-pad only; matmuls never touch horizontal pads.
    # Half-1 cores get vertically FLIPPED inputs so every core sees the
    # same "top-half" structure (zero pad rows at small local indices);
    # the flip negates dy, undone during host reassembly.
    x2pv = np.zeros((B, C, H + 2 * MD, W), dtype=ml_dtypes.bfloat16)
    x2pv[:, :, MD:MD + H, :] = x2

    if _CACHED is None:
        _CACHED = _build_nc()
    nc = _CACHED

    in_maps = []
    for core in range(8):
        b, half = core // 2, core % 2
        if half == 0:
            a = x1[b, :, 0:HH, :]
            x2s = x2pv[b, :, 10:10 + XROWS, :]
        else:
            a = x1[b, :, :HH - 1:-1, :]               # rows 95..48
            x2s = x2pv[b, :, 125:125 - XROWS:-1, :]   # padded 125..48
        in_maps.append({
            "x1h": np.ascontiguousarray(a.reshape(C, HH * W)),
            "x2p": np.ascontiguousarray(x2s.reshape(C, XROWS * W)),
        })

    try:
        res = run_bass_kernel_spmd(
            nc, in_maps, core_ids=list(range(8)),
            trace=os.environ.get("CORR_TRACE") == "1",
        )
    except ImportError:
        res = run_bass_kernel_spmd(nc, in_maps, core_ids=list(range(8)))
    LAST_EXEC_NS = res.exec_time_ns

    out = np.empty((B, CC, OH, OW), dtype=np.float32)
    for core in range(8):
        b, half = core // 2, core % 2
        r = np.asarray(res.results[core]["out"]).reshape(NOH, OW, CC)
        if half == 0:
            out[b, :, 0:NOH, :] = r.transpose(2, 0, 1)
        else:
            # local rp -> global pooled row 47-rp; local dy j -> 40-j
            rr = r.reshape(NOH, OW, K, K)[::-1, :, :, ::-1]
            out[b, :, NOH:2 * NOH, :] = (
                rr.reshape(NOH, OW, CC).transpose(2, 0, 1))
    return out


# revision 101
# speedup vs baseline: 1.2922x; 1.0089x over previous
"""FlowNet-C correlation layer (MAX_DISP=20, STRIDE=2) on 8 trn2 cores.

Strategy: shard by (batch b, output-row half). Core k handles b=k//2,
24 output row-pairs; half-1 cores receive vertically FLIPPED inputs so
every core sees identical "top-half" structure (the flip negates dy,
undone in host reassembly). Contraction over C=128 runs on the
TensorEngine as banded-Gram matmuls in bf16: per row pair (h0,h0+1)
and 5-dy half-group, psum_bank[96, 480] += x1_row[128,96]^T @
x2rows[128,480], with the h-pair pooled via PSUM accumulation (a
matmul output must stay inside one 2KB PSUM bank). Moving columns
span only the 96 REAL x2 columns (no horizontal zero padding) and the
leading all-zero dy group of row-pairs 0-4 is skipped, so PE cost is
~7500 cycles/row-pair instead of 11152.

Each 2-bank 10-dy group is evicted by a single 4D-AP copy into a
[w1, w2, dy] bf16 SBUF tile (dy innermost), alternating DVE/Act
(GPSIMD cannot touch PSUM). One SP DMA per rp writes it into the
middle of a padded [96, 136*41] DRAM row inside one of THREE
persistent pair tiles (pool DRAM tiles get fresh memory per
generation, so the 20-column pads are zeroed once per persistent
tile; the 3-deep cycle also gives a safe write-after-read margin).
In that layout the 1681 wanted (dx, dy) elements of an output pixel
are one contiguous 3362B run, so diagonal extraction for a PAIR of
row-pairs (both w-parities, both rps) is a single thin 4D DMA on the
Pool queue. A DVE add (bf16 2x mode) folds the 2x2 pool; outs are
quad-merged (one DMA per 4 rps, halves staggered on Act/Pool).

Every downstream stage is skewed (gram rp-1, extract pair at -4,
add -5, out -9..) so each DMA's deps are complete at dispatch and the
queues run back-to-back instead of paying ~1.3us DGE setup latency
per DMA. The last two rps use private single-rp gram tiles and a
split gram write so the drain's extract/add/out chains decouple and
overlap. Scale 1/(4*C) is folded into x1 on the host; inputs are
pre-converted to bf16 host-side.
"""

import os

import numpy as np
import ml_dtypes

import concourse.bacc as bacc
import concourse.bass as bass
import concourse.mybir as mybir
import concourse.tile as tile
from concourse.ap import AP
from concourse.bass import MemorySpace
from concourse.bass_utils import run_bass_kernel_spmd

MD = 20
K = 41
CC = K * K            # 1681
B, C, H, W = 4, 128, 96, 96
OH, OW = 48, 48
HH = 48               # full-res rows per core
NOH = 24              # output row-pairs per core
XROWS = 78            # x2 rows per core: 10 host-zeroed + 68 real
                      # (buffer row = local padded row - 10; both halves are
                      # fed "top-half" structure -- half 1 is flipped on the
                      # host -- so padded rows 0-9 are all-zero and never
                      # referenced: groups needing them are skipped)
GW = 96               # gram moving width (real columns only)
GFREE = (W + 2 * MD) * K  # 5576 = padded per-partition gram row in DRAM
PADL = MD * K         # 820 = left pad cols in the DRAM gram row

F32 = mybir.dt.float32
BF16 = mybir.dt.bfloat16

LAST_EXEC_NS = None
_CACHED = None


def _build_nc():
    nc = bacc.Bacc("TRN2", target_bir_lowering=False)
    x1d = nc.dram_tensor("x1h", [C, HH * W], BF16, kind="ExternalInput")
    x2d = nc.dram_tensor("x2p", [C, XROWS * W], BF16, kind="ExternalInput")
    outd = nc.dram_tensor("out", [NOH * OW, CC], BF16, kind="ExternalOutput")

    with tile.TileContext(nc) as tc:
        with (
            tc.tile_pool(name="inp", bufs=1) as inp_pool,
            tc.tile_pool(name="gsb", bufs=3) as gs_pool,
            tc.tile_pool(name="dd", bufs=3) as d_pool,
            tc.tile_pool(name="st", bufs=3) as s_pool,
            tc.tile_pool(name="ps", bufs=3, space=MemorySpace.PSUM) as psum_pool,
            tc.tile_pool(name="pss", bufs=2, space=MemorySpace.PSUM) as psums_pool,
            tc.tile_pool(name="dr", bufs=3, space=MemorySpace.DRAM) as dram_pool,
            tc.tile_pool(name="drs", bufs=2, space=MemorySpace.DRAM) as drams_pool,
        ):
            A = inp_pool.tile([C, HH * W], BF16)
            Bt = inp_pool.tile([C, XROWS * W], BF16)
            Z = inp_pool.tile([96, 4 * PADL], BF16)
            nc.vector.memset(Z[:], 0.0)

            # loads are hand-scheduled: rp0's needs (x1 rows 0-1, x2 rows
            # 0-31) land first split over the three queues; the bulk is
            # injected into early loop bodies after those bodies' critical
            # DMAs so the queues never starve the per-rp pipeline.
            def load(which, r0, n, eng):
                if which == "a":
                    eng.dma_start(A[:, r0 * W:(r0 + n) * W],
                                  x1d[:, r0 * W:(r0 + n) * W])
                else:
                    eng.dma_start(Bt[:, r0 * W:(r0 + n) * W],
                                  x2d[:, r0 * W:(r0 + n) * W])

            load("a", 0, 2, nc.sync)
            load("b", 0, 11, nc.sync)
            load("b", 11, 11, nc.scalar)
            load("b", 22, 12, nc.gpsimd)
            load("a", 2, 8, nc.sync)
            # bulk loads injected into early bodies below:
            bulk_loads = {0: [("b", 34, 22, "gpsimd")],
                          1: [("a", 10, 6, "scalar"),
                              ("b", 56, 22, "gpsimd")],
                          2: [("a", 16, 32, "sync")]}

            # Per-iteration stages are software-pipelined so no engine queue
            # head-of-line blocks: body rp issues extraction for rp-1,
            # pool-add for rp-2 and the output DMA for rp-3 (their inputs
            # completed in earlier iterations), then this rp's Gram write
            # last (it waits on this body's evictions).
            # gram/extract/add are per row-pair; only the OUT stage is
            # merged over pairs of row-pairs (one DMA per 2 rps) to halve
            # its fixed overhead -- that's what lets Act fit its budget.
            gbs, gbs_s, des, ss = {}, {}, {}, {}

            G2 = 2 * GFREE

            def extract(pp, r0=0, r1=2, eng=None):
                # diagonal extraction for pair pp (row-pairs 2pp, 2pp+1
                # stacked in one DRAM tile): element (u, p, r, dx, dy) sits
                # at flat (2u+p)*2*5576 + r*5576 + ((2u+p)+dx)*41 + dy; the
                # (dx, dy) block is one contiguous 1681-elem (3362B) run
                # per (u, p, r) -> one thin 4D DMA per pair.
                gt = gbs[pp][:].tensor
                if pp not in des:
                    des[pp] = d_pool.tile([OW, 2, 2, CC], BF16, tag="de",
                                          name=f"de{pp}")
                shear = [[2 * (G2 + K), OW], [G2 + K, 2],
                         [GFREE, r1 - r0], [1, CC]]
                (eng or nc.gpsimd).dma_start(
                    des[pp][:, :, r0:r1, :], AP(gt, r0 * GFREE, shear))

            gsbs = {}

            # TWO persistent DRAM pair tiles, reused alternately by all
            # pairs (pool tiles get fresh DRAM per generation, so the pad
            # zeroing below would not survive re-allocation). Reusing the
            # same tile objects also gives exact write-after-read ordering
            # from tile's whole-tensor tracking.
            gb2 = [dram_pool.tile([96, G2], BF16, tag="gb", name="gbA"),
                   dram_pool.tile([96, G2], BF16, tag="gb", name="gbB"),
                   dram_pool.tile([96, G2], BF16, tag="gb", name="gbC")]

            def zero_pads(i, eng):
                gt = gb2[i][:].tensor
                for hh in (0, 1):  # 3-dim APs: one per pair half
                    eng.dma_start(
                        AP(gt, hh * GFREE,
                           [[G2, 96], [GFREE - PADL, 2], [1, PADL]]),
                        Z[:, :2 * PADL].rearrange("p (a b) -> p a b", a=2))

            # slot i first read by extract(pair i) at body 2i+4; stagger
            # the zeroing across queues/bodies to keep startup responsive
            zero_pads(0, nc.sync)

            def gram_write(r):
                # Gram (bf16, [w1, w2*41+dy]) -> middle of rp r's half of
                # the pair's padded DRAM tile, one SP DMA; pad columns were
                # zeroed once at startup, never rewritten. The last two rps
                # get their own single tiles.
                if r >= 22:
                    nc.sync.dma_start(
                        gbs_s[r][:, PADL:GFREE - PADL], gsbs[r][:])
                    return
                gbs[r // 2] = Gb = gb2[(r // 2) % 3]
                half = r % 2
                nc.sync.dma_start(
                    Gb[:, half * GFREE + PADL:half * GFREE + GFREE - PADL],
                    gsbs[r][:])

            def extract_single(r, eng):
                # last two row-pairs live in their own single-rp DRAM tiles
                # so each drain extraction depends only on its own gram
                # write (tile deps are whole-tile for raw APs).
                gt = gbs_s[r][:].tensor
                pp = r // 2
                if pp not in des:
                    des[pp] = d_pool.tile([OW, 2, 2, CC], BF16, tag="de",
                                          name=f"de{pp}")
                shear = [[2 * (GFREE + K), OW], [GFREE + K, 2], [1, CC]]
                eng.dma_start(des[pp][:, :, r % 2, :], AP(gt, 0, shear))

            def pool_add(r, eng=None):  # 2x2-pool finish
                q = r // 4
                if q not in ss:
                    ss[q] = s_pool.tile([OW, 4, CC], BF16, tag="s",
                                        name=f"s{q}")
                (eng or nc.vector).tensor_add(
                    ss[q][:, r % 4, :], des[r // 2][:, 0, r % 2, :],
                    des[r // 2][:, 1, r % 2, :])

            def out_dma(q, r0=0, r1=4, eng=None):
                # one DMA per 4 row-pairs; dst row = (4*q + r)*48 + u
                ot = outd[:].tensor
                (eng or nc.scalar).dma_start(
                    AP(ot, (4 * q + r0) * OW * CC,
                       [[CC, OW], [OW * CC, r1 - r0], [1, CC]]),
                    ss[q][:, r0:r1, :])

            # dy-groups (10,10,10,10,1): each 10-group is a [96, 2, 512]
            # 2-bank PSUM tile (a matmul output must stay inside ONE 2KB
            # bank -> 5 dys per matmul), evicted by a single 4D-AP copy.
            # Eviction engines DVE/Act alternating (GPSIMD cannot access
            # PSUM); keeps every engine under the PE's ~3.3us/row-pair.
            ev_engs = [nc.vector.tensor_copy, nc.scalar.copy,
                       nc.vector.tensor_copy, nc.scalar.copy,
                       nc.scalar.copy]

            for rp in range(NOH):
                h0 = 2 * rp
                a0 = A[:, h0 * W:(h0 + 1) * W]
                a1 = A[:, (h0 + 1) * W:(h0 + 2) * W]
                Gsb = gs_pool.tile([96, GW, K], BF16, tag="gsb",
                                   name=f"gsb{rp}")
                gsbs[rp] = Gsb
                if rp < 5:
                    # rps 0-4 skip group g0, so its dy<10 region would
                    # otherwise reach DRAM uninitialized; zero it (also
                    # makes those output channels exactly right already)
                    meng = nc.vector if rp % 2 == 0 else nc.gpsimd
                    meng.memset(Gsb[:, :, 0:10], 0.0)

                # 4 groups of 10 dys + single-dy tail; h-pair pooled via
                # PSUM accumulate (start on hoff 0, stop on hoff 1).
                # Buffer row for (h, dy) is h + dy - 10; rows < 10 of group
                # g0 only exist for rp >= 5, earlier rps skip g0 entirely
                # (channels provably zero, fixed up after the pool-add).
                g_lo = 0 if rp >= 5 else 1
                for g in range(g_lo, 4):
                    d0 = 10 * g
                    ps = psum_pool.tile([96, 2, 512], F32, tag="ps",
                                        name=f"ps{rp}_{g}")
                    for hoff, hap in ((0, a0), (1, a1)):
                        for j in (0, 1):
                            r0 = h0 + hoff + d0 + 5 * j - 10
                            nc.tensor.matmul(
                                ps[:, j, :5 * GW],
                                hap,
                                Bt[:, r0 * W:r0 * W + 5 * GW],
                                start=(hoff == 0), stop=(hoff == 1),
                            )
                    ev_engs[g](
                        Gsb[:, :, d0:d0 + 10].rearrange(
                            "p w (j d) -> p j d w", j=2),
                        ps[:, :, :5 * GW].rearrange(
                            "p j (d w) -> p j d w", w=GW))
                pss = psums_pool.tile([96, GW], F32, tag="ps5",
                                      name=f"pss{rp}")
                for hoff, hap in ((0, a0), (1, a1)):
                    r0 = h0 + hoff + 30
                    nc.tensor.matmul(
                        pss[:], hap, Bt[:, r0 * W:r0 * W + GW],
                        start=(hoff == 0), stop=(hoff == 1),
                    )
                ev_engs[4](Gsb[:, :, 40], pss[:])

                # skewed stages AFTER this body's evictions so they never
                # head-of-line block the eviction dispatches; every DMA's
                # deps are complete at dispatch (gram is skewed one body,
                # extract two, add/out deeper), so each queue runs
                # back-to-back at transfer rate instead of paying the
                # ~1.3us DGE setup latency per DMA.
                if rp >= 1:
                    gram_write(rp - 1)
                if rp == 1:
                    zero_pads(1, nc.gpsimd)
                elif rp == 3:
                    zero_pads(2, nc.scalar)
                if rp % 2 == 0 and rp >= 4:
                    extract((rp - 4) // 2,
                            eng=getattr(nc, os.environ.get("CORR_XE", "gpsimd")))
                if rp >= 5:
                    pool_add(rp - 5)
                om = os.environ.get("CORR_OM", "split2")
                if om in ("pool", "act"):
                    if rp % 4 == 1 and rp >= 9:
                        out_dma((rp - 9) // 4,
                                eng=nc.gpsimd if om == "pool" else nc.scalar)
                else:  # staggered halves on Act and Pool
                    if rp % 4 == 1 and rp >= 9:
                        out_dma((rp - 9) // 4, 0, 2, eng=nc.scalar)
                    if rp % 4 == 3 and rp >= 11:
                        out_dma((rp - 11) // 4, 2, 4, eng=nc.gpsimd)

                # bulk input loads, injected after the early bodies'
                # critical-path work so queues stay responsive
                for which, r0, n, engname in bulk_loads.get(rp, []):
                    load(which, r0, n, getattr(nc, engname))

                # single-rp gram tiles for the last two rps (created and
                # pad-zeroed mid-stream) so the drain chains decouple
                if rp in (4, 6):
                    r_late = 22 + (rp - 4) // 2
                    gbs_s[r_late] = drams_pool.tile(
                        [96, GFREE], BF16, tag="gbs", name=f"gbs{r_late}")
                    nc.scalar.dma_start(
                        AP(gbs_s[r_late][:].tensor, 0,
                           [[GFREE, 96], [GFREE - PADL, 2], [1, PADL]]),
                        Z[:, :2 * PADL].rearrange("p (a b) -> p a b", a=2))

            # drain the pipeline. The critical chain is evict(23) ->
            # gram(23) [split SP+Act in parallel] -> extract(23) [Pool,
            # behind only extract(10)] -> add(23) [DVE] -> out [SP].
            gt23 = gbs_s[23][:].tensor
            for (w0, w1), geng in (((0, 64), nc.sync), ((64, 96), nc.scalar)):
                geng.dma_start(
                    AP(gt23, w0 * GFREE + PADL, [[GFREE, w1 - w0],
                                                 [1, GW * K]]),
                    gsbs[23][w0:w1, :, :])
            extract(10, eng=nc.gpsimd)           # rps 20, 21
            extract_single(22, eng=nc.scalar)    # gram(22) done
            extract_single(23, eng=nc.gpsimd)    # waits gram(23) halves
            pool_add(19)
            out_dma(4, eng=nc.sync)              # rps 16-19
            pool_add(20)
            pool_add(21)
            out_dma(5, 0, 2, eng=nc.scalar)      # rps 20, 21
            pool_add(22)
            out_dma(5, 2, 3, eng=nc.scalar)      # rp 22
            pool_add(23)
            out_dma(5, 3, 4, eng=nc.sync)        # rp 23
    nc.compile()
    return nc


def kernel(x1: np.ndarray, x2: np.ndarray) -> np.ndarray:
    global LAST_EXEC_NS, _CACHED
    x1 = np.asarray(x1, dtype=np.float32) * np.float32(1.0 / (4 * C))
    x1 = x1.astype(ml_dtypes.bfloat16)
    x2 = np.asarray(x2, dtype=np.float32).astype(ml_dtypes.bfloat16)
    # vertical zero-pad only; matmuls never touch horizontal pads.
    # Half-1 cores get vertically FLIPPED inputs so every core sees the
    # same "top-half" structure (zero pad rows at small local indices);
    # the flip negates dy, undone during host reassembly.
    x2pv = np.zeros((B, C, H + 2 * MD, W), dtype=ml_dtypes.bfloat16)
    x2pv[:, :, MD:MD + H, :] = x2

    if _CACHED is None:
        _CACHED = _build_nc()
    nc = _CACHED

    in_maps = []
    for core in range(8):
        b, half = core // 2, core % 2
        if half == 0:
            a = x1[b, :, 0:HH, :]
            x2s = x2pv[b, :, 10:10 + XROWS, :]
        else:
            a = x1[b, :, :HH - 1:-1, :]               # rows 95..48
            x2s = x2pv[b, :, 125:125 - XROWS:-1, :]   # padded 125..48
        in_maps.append({
            "x1h": np.ascontiguousarray(a.reshape(C, HH * W)),
            "x2p": np.ascontiguousarray(x2s.reshape(C, XROWS * W)),
        })

    try:
        res = run_bass_kernel_spmd(
            nc, in_maps, core_ids=list(range(8)),
            trace=os.environ.get("CORR_TRACE") == "1",
        )
    except ImportError:
        res = run_bass_kernel_spmd(nc, in_maps, core_ids=list(range(8)))
    LAST_EXEC_NS = res.exec_time_ns

    out = np.empty((B, CC, OH, OW), dtype=np.float32)
    for core in range(8):
        b, half = core // 2, core % 2
        r = np.asarray(res.results[core]["out"]).reshape(NOH, OW, CC)
        if half == 0:
            out[b, :, 0:NOH, :] = r.transpose(2, 0, 1)
        else:
            # local rp -> global pooled row 47-rp; local dy j -> 40-j
            rr = r.reshape(NOH, OW, K, K)[::-1, :, :, ::-1]
            out[b, :, NOH:2 * NOH, :] = (
                rr.reshape(NOH, OW, CC).transpose(2, 0, 1))
    return out
